# revision 1
# baseline (speedup 1.0000x reference)
"""Trainium2 Bass kernel for nn_BertEncoder_403726926494.

Reference computation (per batch element):
  - ragged sentence extraction from hidden_states, masked-softmax attention
    pooling per sentence with W_doc            -> doc_pooled [B, D, H]
  - query extraction (rows 1..32), masked-softmax pooling with W_query
    broadcast over D                           -> q_bcast   [B, D, H]

Device strategy (SPMD, one program on 8 cores, 8 batch elements per core):
  - Per core-slot, DMA only the used row-span of hidden_states into SBUF
    (slots are assigned from a global sort of spans so the per-slot span is
    a static program constant shared by all cores).
  - Per-token scores s[t] = x_t . W_doc: DVE/GpSimd tensor_tensor multiply
    against a W-broadcast tile, then a free-dim reduce on ACT (activation
    Copy + accum_out) or DVE (tensor_reduce) -- engine choice per slot to
    balance load.
  - softmax without max-subtraction (scores are O(1)):
      alphaU[t,j] = exp(s[t] + logSel[t,j])   one ACT op per chunk, where
    logSel is a host-built {0, -1e30} mask marking token t in sentence j
    (columns padded to 32 with -1e30).
      num[j,:H] | den[j] = alphaU^T @ [X | 1]  PE matmul with a ones-column
    appended to X; 4 slots share one PSUM tile via tile_position col-groups.
  - out[j] = num[j] / (den[j] + eps)  (eps keeps empty sentences at 0).
  - Query path packs 4 examples x 32 query rows onto 128 partitions; the
    query-length mask and example-block structure fold into one host-built
    log-mask. q_pooled is broadcast over D on the host.
  - b_doc / b_query shift every score in a softmax segment equally, so they
    cancel and are ignored.
"""

import numpy as np

B, L, H = 64, 512, 768
D, S, Q = 16, 64, 32
NCORES = 8
SLOTS = 8
MPAD = 32  # selector columns padded to one PE col-group
NEG_BIAS = -1.0e30
DEN_EPS = 1.0e-30

# Engine assignment knobs (tuned from traces):
#   score TT multiply per slot: "dve" or "gps"
#   score reduce per slot: "act" (per-chunk accum) or "dve" (merged reduce)
TT_ENGINE = ["dve"] * SLOTS
RED_ENGINE = ["act", "act", "act", "act", "act", "act", "dve", "dve"]
Q_RED_ENGINE = "act"

_compiled: dict = {}


def _slot_geometry(slot_spans):
    nts = [(sp + 127) // 128 for sp in slot_spans]
    rems = [sp - 128 * (nt - 1) for sp, nt in zip(slot_spans, nts)]
    coffs = [0]
    for nt in nts:
        coffs.append(coffs[-1] + nt)
    return nts, rems, coffs


def _build(slot_spans):
    """Build + compile the SPMD Bass program for the given per-slot spans."""
    from contextlib import ExitStack

    import concourse.bacc as bacc
    import concourse.tile as tile
    from concourse import mybir

    f32 = mybir.dt.float32
    MULT = mybir.AluOpType.mult
    ADD = mybir.AluOpType.add
    EXP = mybir.ActivationFunctionType.Exp
    COPY = mybir.ActivationFunctionType.Copy

    nts, rems, coffs = _slot_geometry(slot_spans)
    ntsum = coffs[-1]
    foffs = [0]
    for nt in nts:
        foffs.append(foffs[-1] + nt - 1)
    roffs = [0]
    for r in rems:
        roffs.append(roffs[-1] + r)

    nc = bacc.Bacc(
        "TRN2", target_bir_lowering=False, debug=False, num_devices=NCORES
    )
    nfull = sum(nt - 1 for nt in nts)
    nremtot = sum(rems)
    sfull = nc.dram_tensor(
        "sfull", [128, max(nfull, 1), H], f32, kind="ExternalInput"
    ).ap()
    srem = nc.dram_tensor("srem", [nremtot, H], f32, kind="ExternalInput").ap()
    qstage = nc.dram_tensor("qstage", [2, 128, H], f32, kind="ExternalInput").ap()
    wd = nc.dram_tensor("wd", [1, H], f32, kind="ExternalInput").ap()
    wq = nc.dram_tensor("wq", [1, H], f32, kind="ExternalInput").ap()
    selt = nc.dram_tensor(
        "selt", [128, ntsum, MPAD], f32, kind="ExternalInput"
    ).ap()
    qmask = nc.dram_tensor("qmask", [128, 2, MPAD], f32, kind="ExternalInput").ap()
    doc_out = nc.dram_tensor("doc_out", [SLOTS, D, H], f32, kind="ExternalOutput").ap()
    q_out = nc.dram_tensor("q_out", [SLOTS, H], f32, kind="ExternalOutput").ap()

    with tile.TileContext(nc) as tc, ExitStack() as ctx:
        const = ctx.enter_context(tc.tile_pool(name="const", bufs=1))

        wrow_d = const.tile([1, H], f32)
        nc.sync.dma_start(out=wrow_d[:], in_=wd[:])
        wrow_q = const.tile([1, H], f32)
        nc.sync.dma_start(out=wrow_q[:], in_=wq[:])
        selt_t = const.tile([128, ntsum, MPAD], f32)
        nc.sync.dma_start(out=selt_t[:], in_=selt[:])
        qmask_t = const.tile([128, 2, MPAD], f32)
        nc.sync.dma_start(out=qmask_t[:], in_=qmask[:])

        # Broadcast W rows across all 128 partitions (gpsimd custom op).
        wb_d = const.tile([128, H], f32)
        wb_q = const.tile([128, H], f32)
        nc.gpsimd.partition_broadcast(wb_d[:], wrow_d[:])
        nc.gpsimd.partition_broadcast(wb_q[:], wrow_q[:])

        xpool = ctx.enter_context(tc.tile_pool(name="xp", bufs=8))
        apool = ctx.enter_context(tc.tile_pool(name="apl", bufs=4))
        scrp = ctx.enter_context(tc.tile_pool(name="scr", bufs=2))
        outp = ctx.enter_context(tc.tile_pool(name="outp", bufs=2))
        smallp = ctx.enter_context(tc.tile_pool(name="smallp", bufs=4))
        qpoolp = ctx.enter_context(tc.tile_pool(name="qpl", bufs=2))
        nump = ctx.enter_context(tc.tile_pool(name="nump", bufs=2, space="PSUM"))
        qnump = ctx.enter_context(tc.tile_pool(name="qnump", bufs=1, space="PSUM"))

        # ---- scores: xw = x * W_bcast (TT), then free-dim reduce -> scol ----
        def emit_scores(x_ap_full, nt, rem, scol, wb, name, tt_eng, red_eng):
            # x_ap_full: [128, nt, H(+1)] view; uses cols 0:H
            xw = scrp.tile([128, nt, H], f32, tag="scratch", name=f"xw{name}")
            tt = nc.gpsimd if tt_eng == "gps" else nc.vector
            if nt > 1:
                tt.tensor_tensor(
                    out=xw[:, 0 : nt - 1, :],
                    in0=x_ap_full[:, 0 : nt - 1, 0:H],
                    in1=wb[:].rearrange("p (o h) -> p o h", o=1).broadcast_to(
                        [128, nt - 1, H]
                    ),
                    op=MULT,
                )
            tt.tensor_tensor(
                out=xw[0:rem, nt - 1, :],
                in0=x_ap_full[0:rem, nt - 1, 0:H],
                in1=wb[0:rem, :],
                op=MULT,
            )
            if red_eng == "dve":
                if nt > 1:
                    nc.vector.tensor_reduce(
                        out=scol[:, 0 : nt - 1],
                        in_=xw[:, 0 : nt - 1, :],
                        axis=mybir.AxisListType.X,
                        op=ADD,
                    )
                nc.vector.tensor_reduce(
                    out=scol[0:rem, nt - 1 : nt],
                    in_=xw[0:rem, nt - 1, :],
                    axis=mybir.AxisListType.X,
                    op=ADD,
                )
            else:
                s2 = scrp.tile([128, H], f32, tag="scratch2", name=f"s2{name}")
                for c in range(nt):
                    cnt = 128 if c < nt - 1 else rem
                    nc.scalar.activation(
                        s2[0:cnt, :], xw[0:cnt, c, :], COPY,
                        bias=0.0, scale=1.0,
                        accum_out=scol[0:cnt, c : c + 1],
                    )

        # ---- doc slots: per-slot pipeline; two groups of 4 share PSUM tiles
        # via PE col-groups. Slots are emitted alternating between the two
        # groups so independent work overlaps and consecutive slots' matmuls
        # land on different col-groups (concurrent PE streams).
        numgs = {}

        xtiles = {}

        def load_slot(s):
            nt, rem = nts[s], rems[s]
            x = xpool.tile([128, nt, H + 1], f32, tag="x", name=f"x{s}")
            if nt > 1:
                nc.sync.dma_start(
                    out=x[:, 0 : nt - 1, 0:H],
                    in_=sfull[:, foffs[s] : foffs[s] + nt - 1, :],
                )
            nc.sync.dma_start(
                out=x[0:rem, nt - 1, 0:H],
                in_=srem[roffs[s] : roffs[s] + rem, :],
            )
            nc.vector.memset(x[:, :, H : H + 1], 1.0)
            xtiles[s] = x

        def emit_slot(s):
            g, k = divmod(s, 4)
            if g not in numgs:
                numgs[g] = nump.tile([128, 1024], f32, tag="num", name=f"num{g}")
            numg = numgs[g]
            nt, rem, coff = nts[s], rems[s], coffs[s]
            x = xtiles[s]

            scol = smallp.tile([128, nt], f32, tag="scol", name=f"scol{s}")
            emit_scores(
                x[:], nt, rem, scol, wb_d, f"d{s}", TT_ENGINE[s], RED_ENGINE[s]
            )

            at = apool.tile([128, nt, MPAD], f32, tag="at", name=f"at{s}")
            for c in range(nt):
                cnt = 128 if c < nt - 1 else rem
                nc.scalar.activation(
                    at[0:cnt, c, :],
                    selt_t[0:cnt, coff + c, :],
                    EXP,
                    bias=scol[0:cnt, c : c + 1],
                    scale=1.0,
                )
            for c in range(nt):
                cnt = 128 if c < nt - 1 else rem
                first, last = c == 0, c == nt - 1
                nc.tensor.matmul(
                    numg[32 * k : 32 * k + MPAD, 0:512],
                    at[0:cnt, c, :],
                    x[0:cnt, c, 0:512],
                    start=first, stop=last,
                    tile_position=(0, 32 * k),
                    skip_group_check=True,
                )
                nc.tensor.matmul(
                    numg[32 * k : 32 * k + MPAD, 512 : H + 1],
                    at[0:cnt, c, :],
                    x[0:cnt, c, 512 : H + 1],
                    start=first, stop=last,
                    tile_position=(0, 32 * k),
                    skip_group_check=True,
                )

        def finish_group(g):
            numg = numgs[g]
            de = smallp.tile([128, 1], f32, tag="de", name=f"de{g}")
            nc.vector.tensor_scalar(
                out=de[:], in0=numg[:, H : H + 1], scalar1=DEN_EPS,
                scalar2=None, op0=ADD,
            )
            rec = smallp.tile([128, 1], f32, tag="rec", name=f"rec{g}")
            nc.vector.reciprocal(rec[:], de[:])
            do = outp.tile([128, H], f32, tag="do", name=f"do{g}")
            nc.scalar.activation(
                do[:], numg[:, 0:H], COPY, bias=0.0, scale=rec[:, 0:1]
            )
            for k in range(4):
                nc.scalar.dma_start(
                    out=doc_out[4 * g + k, :, :],
                    in_=do[32 * k : 32 * k + D, :],
                )

        # ---- query: two batches of 4 examples x 32 rows -> one PSUM tile ----
        def emit_query(qnumg, b):
            qpack = qpoolp.tile([128, H + 1], f32, tag="qpack", name=f"qpack{b}")
            nc.sync.dma_start(out=qpack[:, 0:H], in_=qstage[b, :, :])
            nc.vector.memset(qpack[:, H : H + 1], 1.0)
            qscol = smallp.tile([128, 1], f32, tag="qscol", name=f"qscol{b}")
            emit_scores(
                qpack[:].rearrange("p (o h) -> p o h", o=1), 1, 128, qscol, wb_q,
                f"q{b}", "dve", Q_RED_ENGINE,
            )
            qat = apool.tile([128, MPAD], f32, tag="qat", name=f"qat{b}")
            nc.scalar.activation(
                qat[:], qmask_t[:, b, :], EXP, bias=qscol[:, 0:1], scale=1.0
            )
            nc.tensor.matmul(
                qnumg[32 * b : 32 * b + MPAD, 0:512],
                qat[:], qpack[:, 0:512],
                start=True, stop=True, tile_position=(0, 32 * b),
            )
            nc.tensor.matmul(
                qnumg[32 * b : 32 * b + MPAD, 512 : H + 1],
                qat[:], qpack[:, 512 : H + 1],
                start=True, stop=True, tile_position=(0, 32 * b),
            )

        qnumg = qnump.tile([64, 1024], f32, tag="qnum", name="qnum")
        for s in range(SLOTS):
            load_slot(s)
        for s in (0, 4, 1, 5):
            emit_slot(s)
        emit_query(qnumg, 0)
        for s in (2, 6, 3, 7):
            emit_slot(s)
        emit_query(qnumg, 1)
        finish_group(0)
        finish_group(1)

        qde = smallp.tile([64, 1], f32, tag="qde", name="qde")
        nc.vector.tensor_scalar(
            out=qde[:], in0=qnumg[:, H : H + 1], scalar1=DEN_EPS,
            scalar2=None, op0=ADD,
        )
        qrec = smallp.tile([64, 1], f32, tag="qrec", name="qrec")
        nc.vector.reciprocal(qrec[:], qde[:])
        qo = outp.tile([64, H], f32, tag="qo", name="qo")
        nc.scalar.activation(
            qo[:], qnumg[:, 0:H], COPY, bias=0.0, scale=qrec[:, 0:1]
        )
        for b in range(2):
            nc.sync.dma_start(
                out=q_out[4 * b : 4 * b + 4, :],
                in_=qo[32 * b : 32 * b + 4, :],
            )

    nc.compile()
    return nc


def _prepare(query_len, seq_lens):
    """Host-side geometry: spans, slot assignment, selector/mask arrays."""
    ql = np.asarray(query_len).astype(np.int64)
    sl = np.asarray(seq_lens).astype(np.int64)
    offs = ql[:, None] + 2 + np.cumsum(sl, axis=1) - sl  # [B, D] sentence starts
    end = ql + 2 + sl.sum(axis=1)
    span = np.maximum(end, 1 + Q)  # query rows 1..32 must be covered
    order = np.argsort(-span, kind="stable")  # rank -> example id
    slot_spans = tuple(int(span[order[8 * s]]) for s in range(SLOTS))
    nts, rems, coffs = _slot_geometry(slot_spans)
    ntsum = coffs[-1]

    selt_all = np.full((NCORES, 128, ntsum, MPAD), NEG_BIAS, np.float32)
    qmask_all = np.full((NCORES, 128, 2, MPAD), NEG_BIAS, np.float32)
    ex_map = np.empty((NCORES, SLOTS), np.int64)
    for c in range(NCORES):
        for s in range(SLOTS):
            e = int(order[8 * s + c])
            ex_map[c, s] = e
            for j in range(D):
                ln = int(sl[e, j])
                if ln == 0:
                    continue
                o = int(offs[e, j])
                t = np.arange(o, o + ln)
                selt_all[c, t % 128, coffs[s] + t // 128, j] = 0.0
            b, sub = divmod(s, 4)
            qmask_all[c, 32 * sub : 32 * sub + int(ql[e]), b, sub] = 0.0
    return slot_spans, ex_map, selt_all, qmask_all


def kernel(hidden_states, W_doc, b_doc, W_query, b_query, query_len, seq_lens):
    hs = np.ascontiguousarray(np.asarray(hidden_states, dtype=np.float32))
    wd = np.ascontiguousarray(np.asarray(W_doc, np.float32).reshape(1, H))
    wq = np.ascontiguousarray(np.asarray(W_query, np.float32).reshape(1, H))

    slot_spans, ex_map, selt_all, qmask_all = _prepare(query_len, seq_lens)

    nc = _compiled.get(slot_spans)
    if nc is None:
        nc = _build(slot_spans)
        _compiled[slot_spans] = nc

    nts, rems, _ = _slot_geometry(slot_spans)
    nfull = sum(nt - 1 for nt in nts)
    nremtot = sum(rems)

    in_maps = []
    for c in range(NCORES):
        sfull = np.empty((128, max(nfull, 1), H), np.float32)
        srem = np.empty((nremtot, H), np.float32)
        qstage = np.empty((2, 128, H), np.float32)
        fo = ro = 0
        for s in range(SLOTS):
            e = int(ex_map[c, s])
            nt, rem = nts[s], rems[s]
            if nt > 1:
                sfull[:, fo : fo + nt - 1, :] = (
                    hs[e, 0 : (nt - 1) * 128, :]
                    .reshape(nt - 1, 128, H)
                    .transpose(1, 0, 2)
                )
                fo += nt - 1
            srem[ro : ro + rem] = hs[e, (nt - 1) * 128 : (nt - 1) * 128 + rem, :]
            ro += rem
            b, sub = divmod(s, 4)
            qstage[b, 32 * sub : 32 * sub + 32, :] = hs[e, 1 : 1 + Q, :]
        in_maps.append(
            {
                "sfull": sfull,
                "srem": srem,
                "qstage": qstage,
                "wd": wd,
                "wq": wq,
                "selt": selt_all[c],
                "qmask": qmask_all[c],
            }
        )

    from concourse.bass_utils import run_bass_kernel_spmd

    res = run_bass_kernel_spmd(nc, in_maps, list(range(NCORES)))

    doc = np.empty((B, D, H), np.float32)
    qp = np.empty((B, H), np.float32)
    for c in range(NCORES):
        r = res.results[c]
        for s in range(SLOTS):
            e = int(ex_map[c, s])
            doc[e] = r["doc_out"][s]
            qp[e] = r["q_out"][s]
    q_bcast = np.broadcast_to(qp[:, None, :], (B, D, H))
    return doc, q_bcast



# revision 6
# speedup vs baseline: 1.4215x; 1.4215x over previous
"""Trainium2 Bass kernel for nn_BertEncoder_403726926494.

Reference computation (per batch element):
  - ragged sentence extraction from hidden_states, masked-softmax attention
    pooling per sentence with W_doc            -> doc_pooled [B, D, H]
  - query extraction (rows 1..32), masked-softmax pooling with W_query
    broadcast over D                           -> q_bcast   [B, D, H]

Device strategy (SPMD, one program on 8 cores, 8 batch elements per core):
  - All float staging in bf16 (tolerance 2e-2 >> bf16 error ~1e-3); PSUM
    accumulation and outputs stay f32.
  - Host packs one dram tensor xall[128, NCH, H+1] per core: doc slots
    (sorted spans, chunked by 128 tokens, zero-padded) followed by 2 query
    chunks (4 examples x 32 rows each), with a ones column at H for the
    softmax denominators.  One DMA per slot, fully contiguous per
    partition.
  - Scores s[t] = x_t . W via one fused DVE tensor_tensor_reduce per chunk.
  - alpha = exp(s + logmask) via one ACT EXP per chunk (logmask in bf16,
    chunk-contiguous), output bf16 -> matmul lhsT.
  - num[j,:H] | den[j] = alpha^T @ [X | 1] PE matmuls in bf16; 4 slots per
    PSUM tile via tile_position col-groups.  The query chunks append to the
    k=3 accumulation chain using selector columns 16..19, so q_pooled lands
    in spare PSUM rows 112..115 and rides the same normalize + store.
  - out = num / (den + eps); one [128, H] f32 store per group; group 1 is
    computed first so its store overlaps group 0 compute.
  - b_doc / b_query shift every score in a softmax segment equally, so they
    cancel and are ignored.
"""

import numpy as np
import ml_dtypes

B, L, H = 64, 512, 768
D, S, Q = 16, 64, 32
NCORES = 8
SLOTS = 8
MPAD = 32          # selector columns per chunk (16 doc sentences + query/spare)
NEG_BIAS = -1.0e30
DEN_EPS = 1.0e-30
BF16 = ml_dtypes.bfloat16

# score engine per slot: "dve" (fused STT) or "gps" (GpSimd mult + ACT reduce)
SCORE_ENG = ["dve", "dve", "gps", "dve", "dve", "gps", "dve", "dve"]

_compiled: dict = {}


def _slot_geometry(slot_spans):
    nts = [(sp + 127) // 128 for sp in slot_spans]
    coffs = [0]
    for nt in nts:
        coffs.append(coffs[-1] + nt)
    return nts, coffs


def _build(slot_spans):
    """Build + compile the SPMD Bass program for the given per-slot spans."""
    from contextlib import ExitStack

    import concourse.bacc as bacc
    import concourse.tile as tile
    from concourse import mybir

    f32 = mybir.dt.float32
    bf16 = mybir.dt.bfloat16
    MULT = mybir.AluOpType.mult
    ADD = mybir.AluOpType.add
    EXP = mybir.ActivationFunctionType.Exp
    COPY = mybir.ActivationFunctionType.Copy
    IDENT = mybir.ActivationFunctionType.Identity

    nts, coffs = _slot_geometry(slot_spans)
    ntsum = coffs[-1]
    NCH = ntsum + 2            # + two query chunks
    QC = [ntsum, ntsum + 1]    # query chunk index for group 0 / group 1

    nc = bacc.Bacc(
        "TRN2", target_bir_lowering=False, debug=False, num_devices=NCORES
    )
    xall = nc.dram_tensor("xall", [128, NCH, H + 1], bf16, kind="ExternalInput").ap()
    sel = nc.dram_tensor("sel", [128, NCH, MPAD], bf16, kind="ExternalInput").ap()
    wd = nc.dram_tensor("wd", [1, H], bf16, kind="ExternalInput").ap()
    wq = nc.dram_tensor("wq", [1, H], bf16, kind="ExternalInput").ap()
    out = nc.dram_tensor("out", [2, 128, H], f32, kind="ExternalOutput").ap()

    with tile.TileContext(nc) as tc, ExitStack() as ctx:
        const = ctx.enter_context(tc.tile_pool(name="const", bufs=1))
        nump = ctx.enter_context(tc.tile_pool(name="nump", bufs=2, space="PSUM"))

        # --- tiny loads + selector masks on the gpsimd (SWDGE) queue ---
        wrow_d = const.tile([1, H], bf16)
        wrow_q = const.tile([1, H], bf16)
        nc.gpsimd.dma_start(out=wrow_d[:], in_=wd[:])
        nc.gpsimd.dma_start(out=wrow_q[:], in_=wq[:])
        sel_t = const.tile([128, NCH, MPAD], bf16)
        nc.gpsimd.dma_start(out=sel_t[:], in_=sel[:])
        wb_d = const.tile([128, H], bf16)
        wb_q = const.tile([128, H], bf16)
        nc.gpsimd.partition_broadcast(wb_d[:], wrow_d[:])
        nc.gpsimd.partition_broadcast(wb_q[:], wrow_q[:])

        # --- x slot loads: group 1 on scalar queue, group 0 on sync ---
        xt = {}
        for s in range(SLOTS):
            xt[s] = const.tile([128, nts[s], H + 1], bf16, name=f"x{s}")
        xqt = const.tile([128, 2, H + 1], bf16, name="xq")

        def load_slot(s, eng):
            eng.dma_start(
                out=xt[s][:], in_=xall[:, coffs[s] : coffs[s] + nts[s], :]
            )

        load_slot(7, nc.scalar)
        nc.scalar.dma_start(out=xqt[:], in_=xall[:, ntsum : ntsum + 2, :])
        load_slot(6, nc.scalar)
        load_slot(5, nc.scalar)
        load_slot(4, nc.scalar)
        load_slot(3, nc.sync)
        load_slot(2, nc.sync)
        load_slot(1, nc.sync)
        load_slot(0, nc.sync)

        scol = const.tile([128, NCH], f32)
        at = const.tile([128, NCH, MPAD], bf16)
        scratch = const.tile([128, H], bf16)
        s2 = const.tile([128, H], bf16)
        xwp = ctx.enter_context(tc.tile_pool(name="xwp", bufs=2))
        numg = [
            nump.tile([128, 1024], f32, tag="num", name=f"num{g}") for g in range(2)
        ]

        def emit_chunk_scores(x_ap, wb, cc):
            # fused multiply+reduce on DVE: scol[:, cc] = x . W
            nc.vector.scalar_tensor_tensor(
                out=scratch[:],
                in0=x_ap,
                scalar=1.0,
                in1=wb[:],
                op0=MULT,
                op1=MULT,
                accum_out=scol[:, cc : cc + 1],
            )

        def emit_slot_scores_gps(s, wb):
            # whole-slot multiply on GpSimd, per-chunk accum-reduce on ACT
            nt = nts[s]
            xw = xwp.tile([128, nt, H], bf16, tag="xw", name=f"xw{s}")
            nc.gpsimd.tensor_tensor(
                out=xw[:],
                in0=xt[s][:, :, 0:H],
                in1=wb[:].rearrange("p (o h) -> p o h", o=1).broadcast_to(
                    [128, nt, H]
                ),
                op=MULT,
            )
            for c in range(nt):
                nc.scalar.activation(
                    s2[:], xw[:, c, :], COPY, bias=0.0, scale=1.0,
                    accum_out=scol[:, coffs[s] + c : coffs[s] + c + 1],
                )

        def emit_chunk_alpha(cc):
            nc.scalar.activation(
                at[:, cc, :],
                sel_t[:, cc, :],
                EXP,
                bias=scol[:, cc : cc + 1],
                scale=1.0,
            )

        def emit_chunk_matmuls(x_ap, cc, g, k, start, stop):
            nc.tensor.matmul(
                numg[g][32 * k : 32 * k + MPAD, 0:512],
                at[:, cc, :],
                x_ap[:, 0:512],
                start=start, stop=stop,
                tile_position=(0, 32 * k),
                skip_group_check=True,
            )
            nc.tensor.matmul(
                numg[g][32 * k : 32 * k + MPAD, 512 : H + 1],
                at[:, cc, :],
                x_ap[:, 512 : H + 1],
                start=start, stop=stop,
                tile_position=(0, 32 * k),
                skip_group_check=True,
            )

        def emit_slot(s):
            g, k = divmod(s, 4)
            nt = nts[s]
            if SCORE_ENG[s] == "gps":
                emit_slot_scores_gps(s, wb_d)
            for c in range(nt):
                cc = coffs[s] + c
                if SCORE_ENG[s] == "dve":
                    emit_chunk_scores(xt[s][:, c, 0:H], wb_d, cc)
                emit_chunk_alpha(cc)
                # k=3 chain keeps accumulating: the query chunk closes it
                stop = (c == nt - 1) and (k != 3)
                emit_chunk_matmuls(xt[s][:, c, :], cc, g, k, c == 0, stop)

        def emit_query(g):
            cc = QC[g]
            emit_chunk_scores(xqt[:, g, 0:H], wb_q, cc)
            emit_chunk_alpha(cc)
            emit_chunk_matmuls(xqt[:, g, :], cc, g, 3, False, True)

        def finish_group(g, eng):
            de = const.tile([128, 1], f32, name=f"de{g}")
            nc.vector.tensor_scalar(
                out=de[:], in0=numg[g][:, H : H + 1], scalar1=DEN_EPS,
                scalar2=None, op0=ADD,
            )
            rec = const.tile([128, 1], f32, name=f"rec{g}")
            nc.vector.reciprocal(rec[:], de[:])
            do = const.tile([128, H], f32, name=f"do{g}")
            nc.scalar.activation(
                do[:], numg[g][:, 0:H], COPY, bias=0.0, scale=rec[:, 0:1]
            )
            eng.dma_start(out=out[g, :, :], in_=do[:])

        # group 1 first: its store overlaps group 0 compute
        emit_slot(7)
        emit_query(1)
        emit_slot(6)
        emit_slot(5)
        emit_slot(4)
        finish_group(1, nc.scalar)
        emit_slot(3)
        emit_query(0)
        emit_slot(2)
        emit_slot(1)
        emit_slot(0)
        finish_group(0, nc.sync)

    nc.compile()
    return nc


def _prepare(query_len, seq_lens):
    """Host-side geometry: spans, slot assignment, selector mask array."""
    ql = np.asarray(query_len).astype(np.int64)
    sl = np.asarray(seq_lens).astype(np.int64)
    offs = ql[:, None] + 2 + np.cumsum(sl, axis=1) - sl  # [B, D] sentence starts
    end = ql + 2 + sl.sum(axis=1)
    span = np.maximum(end, 1 + Q)  # query rows 1..32 must be covered
    order = np.argsort(-span, kind="stable")  # rank -> example id
    slot_spans = tuple(int(span[order[8 * s]]) for s in range(SLOTS))
    nts, coffs = _slot_geometry(slot_spans)
    ntsum = coffs[-1]

    sel_all = np.full((NCORES, 128, ntsum + 2, MPAD), NEG_BIAS, np.float32)
    ex_map = np.empty((NCORES, SLOTS), np.int64)
    for c in range(NCORES):
        for s in range(SLOTS):
            e = int(order[8 * s + c])
            ex_map[c, s] = e
            for j in range(D):
                ln = int(sl[e, j])
                if ln == 0:
                    continue
                o = int(offs[e, j])
                t = np.arange(o, o + ln)
                sel_all[c, t % 128, coffs[s] + t // 128, j] = 0.0
            g, k = divmod(s, 4)
            sel_all[c, 32 * k : 32 * k + int(ql[e]), ntsum + g, 16 + k] = 0.0
    return slot_spans, ex_map, sel_all


def kernel(hidden_states, W_doc, b_doc, W_query, b_query, query_len, seq_lens):
    hs = np.asarray(hidden_states, dtype=np.float32)
    wd = np.asarray(W_doc, np.float32).reshape(1, H).astype(BF16)
    wq = np.asarray(W_query, np.float32).reshape(1, H).astype(BF16)

    slot_spans, ex_map, sel_all = _prepare(query_len, seq_lens)

    nc = _compiled.get(slot_spans)
    if nc is None:
        nc = _build(slot_spans)
        _compiled[slot_spans] = nc

    nts, coffs = _slot_geometry(slot_spans)
    ntsum = coffs[-1]
    NCH = ntsum + 2

    in_maps = []
    for c in range(NCORES):
        xbuf = np.zeros((128, NCH, H + 1), np.float32)
        xbuf[:, :, H] = 1.0
        for s in range(SLOTS):
            e = int(ex_map[c, s])
            nt, sp = nts[s], slot_spans[s]
            rows = np.zeros((nt * 128, H), np.float32)
            rows[:sp] = hs[e, :sp]
            xbuf[:, coffs[s] : coffs[s] + nt, 0:H] = (
                rows.reshape(nt, 128, H).transpose(1, 0, 2)
            )
            g, k = divmod(s, 4)
            xbuf[32 * k : 32 * k + 32, ntsum + g, 0:H] = hs[e, 1 : 1 + Q]
        in_maps.append(
            {
                "xall": xbuf.astype(BF16),
                "sel": sel_all[c].astype(BF16),
                "wd": wd,
                "wq": wq,
            }
        )

    from concourse.bass_utils import run_bass_kernel_spmd

    res = run_bass_kernel_spmd(nc, in_maps, list(range(NCORES)))

    doc = np.empty((B, D, H), np.float32)
    qp = np.empty((B, H), np.float32)
    for c in range(NCORES):
        r = res.results[c]
        for s in range(SLOTS):
            e = int(ex_map[c, s])
            g, k = divmod(s, 4)
            doc[e] = r["out"][g, 32 * k : 32 * k + D, :]
            qp[e] = r["out"][g, 112 + k, :]
    q_bcast = np.broadcast_to(qp[:, None, :], (B, D, H))
    return doc, q_bcast


# revision 7
# speedup vs baseline: 1.8693x; 1.3150x over previous
"""Trainium2 Bass kernel for nn_BertEncoder_403726926494.

Reference computation (per batch element):
  - ragged sentence extraction from hidden_states, masked-softmax attention
    pooling per sentence with W_doc            -> doc_pooled [B, D, H]
  - query extraction (rows 1..32), masked-softmax pooling with W_query
    broadcast over D                           -> q_bcast   [B, D, H]

Device strategy (SPMD, one program on 8 cores, 8 batch elements per core):
  - All float staging in bf16 (tolerance 2e-2 >> bf16 error ~1e-3); PSUM
    accumulation and outputs stay f32.
  - Host packs one dram tensor xall[128, NCH, H+1] per core: doc slots
    (sorted spans, chunked by 128 tokens, zero-padded) followed by 2 query
    chunks (4 examples x 32 rows each), with a ones column at H for the
    softmax denominators.  One DMA per slot, fully contiguous per
    partition.
  - Scores s[t] = x_t . W via one fused DVE tensor_tensor_reduce per chunk.
  - alpha = exp(s + logmask) via one ACT EXP per chunk (logmask in bf16,
    chunk-contiguous), output bf16 -> matmul lhsT.
  - num[j,:H] | den[j] = alpha^T @ [X | 1] PE matmuls in bf16; 4 slots per
    PSUM tile via tile_position col-groups.  The query chunks append to the
    k=3 accumulation chain using selector columns 16..19, so q_pooled lands
    in spare PSUM rows 112..115 and rides the same normalize + store.
  - out = num / (den + eps); one [128, H] f32 store per group; group 1 is
    computed first so its store overlaps group 0 compute.
  - b_doc / b_query shift every score in a softmax segment equally, so they
    cancel and are ignored.
"""

import numpy as np
import ml_dtypes

B, L, H = 64, 512, 768
D, S, Q = 16, 64, 32
NCORES = 8
SLOTS = 8
MPAD = 32          # selector columns per chunk (16 doc sentences + query/spare)
NEG_BIAS = -1.0e30
DEN_EPS = 1.0e-30
BF16 = ml_dtypes.bfloat16

# score engine per slot: "dve" (fused STT) or "gps" (GpSimd mult + ACT reduce)
SCORE_ENG = ["dve", "dve", "gps", "dve", "dve", "gps", "mix", "dve"]

_compiled: dict = {}


def _slot_geometry(slot_spans):
    nts = [(sp + 127) // 128 for sp in slot_spans]
    coffs = [0]
    for nt in nts:
        coffs.append(coffs[-1] + nt)
    return nts, coffs


def _build(slot_spans):
    """Build + compile the SPMD Bass program for the given per-slot spans."""
    from contextlib import ExitStack

    import concourse.bacc as bacc
    import concourse.tile as tile
    from concourse import mybir

    f32 = mybir.dt.float32
    bf16 = mybir.dt.bfloat16
    MULT = mybir.AluOpType.mult
    ADD = mybir.AluOpType.add
    EXP = mybir.ActivationFunctionType.Exp
    COPY = mybir.ActivationFunctionType.Copy
    IDENT = mybir.ActivationFunctionType.Identity

    nts, coffs = _slot_geometry(slot_spans)
    ntsum = coffs[-1]
    NCH = ntsum + 2            # + two query chunks
    QC = [ntsum, ntsum + 1]    # query chunk index for group 0 / group 1

    nc = bacc.Bacc(
        "TRN2", target_bir_lowering=False, debug=False, num_devices=NCORES
    )
    xall = nc.dram_tensor("xall", [128, NCH, H + 2], bf16, kind="ExternalInput").ap()
    sel = nc.dram_tensor("sel", [128, NCH, MPAD], bf16, kind="ExternalInput").ap()
    wd = nc.dram_tensor("wd", [1, H], bf16, kind="ExternalInput").ap()
    wq = nc.dram_tensor("wq", [1, H], bf16, kind="ExternalInput").ap()
    out = nc.dram_tensor("out", [2, 128, H], f32, kind="ExternalOutput").ap()

    with tile.TileContext(nc) as tc, ExitStack() as ctx:
        const = ctx.enter_context(tc.tile_pool(name="const", bufs=1))
        nump = ctx.enter_context(tc.tile_pool(name="nump", bufs=2, space="PSUM"))

        # --- tiny loads + selector masks on the gpsimd (SWDGE) queue ---
        wrow_d = const.tile([1, H], bf16)
        wrow_q = const.tile([1, H], bf16)
        nc.sync.dma_start(out=wrow_d[:], in_=wd[:])
        nc.sync.dma_start(out=wrow_q[:], in_=wq[:])
        sel_t = const.tile([128, NCH, MPAD], bf16)
        nc.sync.dma_start(out=sel_t[:], in_=sel[:])
        wb_d = const.tile([128, H], bf16)
        wb_q = const.tile([128, H], bf16)
        nc.gpsimd.partition_broadcast(wb_d[:], wrow_d[:])
        nc.gpsimd.partition_broadcast(wb_q[:], wrow_q[:])

        # --- x slot loads: group 1 on scalar queue, group 0 on sync ---
        xt = {}
        for s in range(SLOTS):
            xt[s] = const.tile([128, nts[s], H + 2], bf16, name=f"x{s}")
        xqt = const.tile([128, 2, H + 2], bf16, name="xq")

        def load_slot(s, eng):
            eng.dma_start(
                out=xt[s][:], in_=xall[:, coffs[s] : coffs[s] + nts[s], :]
            )

        load_slot(7, nc.sync)
        load_slot(6, nc.scalar)
        nc.sync.dma_start(out=xqt[:], in_=xall[:, ntsum : ntsum + 2, :])
        load_slot(4, nc.scalar)
        load_slot(5, nc.sync)
        load_slot(2, nc.scalar)
        load_slot(3, nc.sync)
        load_slot(0, nc.scalar)
        load_slot(1, nc.sync)

        scol = const.tile([128, NCH], f32)
        at = const.tile([128, NCH, MPAD], bf16)
        scratch = const.tile([128, H], bf16)
        s2 = const.tile([128, H], bf16)
        xwp = ctx.enter_context(tc.tile_pool(name="xwp", bufs=2))
        numg = [
            nump.tile([128, 1024], f32, tag="num", name=f"num{g}") for g in range(2)
        ]

        def emit_chunk_scores(x_ap, wb, cc):
            # fused multiply+reduce on DVE: scol[:, cc] = x . W
            nc.vector.scalar_tensor_tensor(
                out=scratch[:],
                in0=x_ap,
                scalar=1.0,
                in1=wb[:],
                op0=MULT,
                op1=MULT,
                accum_out=scol[:, cc : cc + 1],
            )

        def emit_slot_scores_mix(s, wb):
            nt = nts[s]
            xw = xwp.tile([128, nt, H], bf16, tag="xw", name=f"xwm{s}")
            nc.vector.tensor_tensor(
                out=xw[:],
                in0=xt[s][:, :, 0:H],
                in1=wb[:].rearrange("p (o h) -> p o h", o=1).broadcast_to(
                    [128, nt, H]
                ),
                op=MULT,
            )
            for c in range(nt):
                nc.scalar.activation(
                    s2[:], xw[:, c, :], COPY, bias=0.0, scale=1.0,
                    accum_out=scol[:, coffs[s] + c : coffs[s] + c + 1],
                )

        def emit_slot_scores_gps(s, wb):
            # whole-slot multiply on GpSimd, per-chunk accum-reduce on ACT
            nt = nts[s]
            xw = xwp.tile([128, nt, H], bf16, tag="xw", name=f"xw{s}")
            nc.gpsimd.tensor_tensor(
                out=xw[:],
                in0=xt[s][:, :, 0:H],
                in1=wb[:].rearrange("p (o h) -> p o h", o=1).broadcast_to(
                    [128, nt, H]
                ),
                op=MULT,
            )
            for c in range(nt):
                nc.scalar.activation(
                    s2[:], xw[:, c, :], COPY, bias=0.0, scale=1.0,
                    accum_out=scol[:, coffs[s] + c : coffs[s] + c + 1],
                )

        def emit_chunk_alpha(cc):
            nc.scalar.activation(
                at[:, cc, :],
                sel_t[:, cc, :],
                EXP,
                bias=scol[:, cc : cc + 1],
                scale=1.0,
            )

        def emit_chunk_matmuls(x_ap, cc, g, k, start, stop):
            nc.tensor.matmul(
                numg[g][32 * k : 32 * k + MPAD, 0:512],
                at[:, cc, :],
                x_ap[:, 0:512],
                start=start, stop=stop,
                tile_position=(0, 32 * k),
                skip_group_check=True,
            )
            nc.tensor.matmul(
                numg[g][32 * k : 32 * k + MPAD, 512 : H + 1],
                at[:, cc, :],
                x_ap[:, 512 : H + 1],
                start=start, stop=stop,
                tile_position=(0, 32 * k),
                skip_group_check=True,
            )

        def emit_slot(s):
            g, k = divmod(s, 4)
            nt = nts[s]
            if SCORE_ENG[s] == "gps":
                emit_slot_scores_gps(s, wb_d)
            elif SCORE_ENG[s] == "mix":
                emit_slot_scores_mix(s, wb_d)
            for c in range(nt):
                cc = coffs[s] + c
                if SCORE_ENG[s] == "dve":
                    emit_chunk_scores(xt[s][:, c, 0:H], wb_d, cc)
                emit_chunk_alpha(cc)
                # k=3 chain keeps accumulating: the query chunk closes it
                stop = (c == nt - 1) and (k != 3)
                emit_chunk_matmuls(xt[s][:, c, :], cc, g, k, c == 0, stop)

        def emit_query(g):
            cc = QC[g]
            emit_chunk_scores(xqt[:, g, 0:H], wb_q, cc)
            emit_chunk_alpha(cc)
            emit_chunk_matmuls(xqt[:, g, :], cc, g, 3, False, True)

        def finish_group(g, eng):
            de = const.tile([128, 1], f32, name=f"de{g}")
            nc.vector.tensor_scalar(
                out=de[:], in0=numg[g][:, H : H + 1], scalar1=DEN_EPS,
                scalar2=None, op0=ADD,
            )
            rec = const.tile([128, 1], f32, name=f"rec{g}")
            nc.vector.reciprocal(rec[:], de[:])
            do = const.tile([128, H], f32, name=f"do{g}")
            nc.scalar.activation(
                do[:], numg[g][:, 0:H], COPY, bias=0.0, scale=rec[:, 0:1]
            )
            eng.dma_start(out=out[g, :, :], in_=do[:])

        # group 1 first: its store overlaps group 0 compute
        emit_slot(7)
        emit_query(1)
        emit_slot(6)
        emit_slot(5)
        emit_slot(4)
        finish_group(1, nc.scalar)
        emit_slot(3)
        emit_query(0)
        emit_slot(2)
        emit_slot(1)
        emit_slot(0)
        finish_group(0, nc.sync)

    nc.compile()
    return nc


def _prepare(query_len, seq_lens):
    """Host-side geometry: spans, slot assignment, selector mask array."""
    ql = np.asarray(query_len).astype(np.int64)
    sl = np.asarray(seq_lens).astype(np.int64)
    offs = ql[:, None] + 2 + np.cumsum(sl, axis=1) - sl  # [B, D] sentence starts
    end = ql + 2 + sl.sum(axis=1)
    span = np.maximum(end, 1 + Q)  # query rows 1..32 must be covered
    order = np.argsort(-span, kind="stable")  # rank -> example id
    slot_spans = tuple(int(span[order[8 * s]]) for s in range(SLOTS))
    nts, coffs = _slot_geometry(slot_spans)
    ntsum = coffs[-1]

    sel_all = np.full((NCORES, 128, ntsum + 2, MPAD), NEG_BIAS, np.float32)
    ex_map = np.empty((NCORES, SLOTS), np.int64)
    for c in range(NCORES):
        for s in range(SLOTS):
            e = int(order[8 * s + c])
            ex_map[c, s] = e
            for j in range(D):
                ln = int(sl[e, j])
                if ln == 0:
                    continue
                o = int(offs[e, j])
                t = np.arange(o, o + ln)
                sel_all[c, t % 128, coffs[s] + t // 128, j] = 0.0
            g, k = divmod(s, 4)
            sel_all[c, 32 * k : 32 * k + int(ql[e]), ntsum + g, 16 + k] = 0.0
    return slot_spans, ex_map, sel_all


def kernel(hidden_states, W_doc, b_doc, W_query, b_query, query_len, seq_lens):
    hs = np.asarray(hidden_states, dtype=np.float32)
    wd = np.asarray(W_doc, np.float32).reshape(1, H).astype(BF16)
    wq = np.asarray(W_query, np.float32).reshape(1, H).astype(BF16)

    slot_spans, ex_map, sel_all = _prepare(query_len, seq_lens)

    nc = _compiled.get(slot_spans)
    if nc is None:
        nc = _build(slot_spans)
        _compiled[slot_spans] = nc

    nts, coffs = _slot_geometry(slot_spans)
    ntsum = coffs[-1]
    NCH = ntsum + 2

    in_maps = []
    for c in range(NCORES):
        xbuf = np.zeros((128, NCH, H + 2), np.float32)
        xbuf[:, :, H] = 1.0
        for s in range(SLOTS):
            e = int(ex_map[c, s])
            nt, sp = nts[s], slot_spans[s]
            rows = np.zeros((nt * 128, H), np.float32)
            rows[:sp] = hs[e, :sp]
            xbuf[:, coffs[s] : coffs[s] + nt, 0:H] = (
                rows.reshape(nt, 128, H).transpose(1, 0, 2)
            )
            g, k = divmod(s, 4)
            xbuf[32 * k : 32 * k + 32, ntsum + g, 0:H] = hs[e, 1 : 1 + Q]
        in_maps.append(
            {
                "xall": xbuf.astype(BF16),
                "sel": sel_all[c].astype(BF16),
                "wd": wd,
                "wq": wq,
            }
        )

    from concourse.bass_utils import run_bass_kernel_spmd

    res = run_bass_kernel_spmd(nc, in_maps, list(range(NCORES)))

    doc = np.empty((B, D, H), np.float32)
    qp = np.empty((B, H), np.float32)
    for c in range(NCORES):
        r = res.results[c]
        for s in range(SLOTS):
            e = int(ex_map[c, s])
            g, k = divmod(s, 4)
            doc[e] = r["out"][g, 32 * k : 32 * k + D, :]
            qp[e] = r["out"][g, 112 + k, :]
    q_bcast = np.broadcast_to(qp[:, None, :], (B, D, H))
    return doc, q_bcast


# revision 8
# speedup vs baseline: 1.9369x; 1.0361x over previous
"""Trainium2 Bass kernel for nn_BertEncoder_403726926494.

Reference computation (per batch element):
  - ragged sentence extraction from hidden_states, masked-softmax attention
    pooling per sentence with W_doc            -> doc_pooled [B, D, H]
  - query extraction (rows 1..32), masked-softmax pooling with W_query
    broadcast over D                           -> q_bcast   [B, D, H]

Device strategy (SPMD, one program on 8 cores, 8 batch elements per core):
  - All float staging in bf16 (tolerance 2e-2 >> bf16 error ~1e-3); PSUM
    accumulation and outputs stay f32.
  - TWO examples share each token stream (A rows then B rows) so the
    128-token chunks carry almost no padding: A's sentences use selector
    columns 0..15, B's use 16..31.  4 paired slots x 8 cores = 32 pairs,
    13 doc chunks/core (vs 17 unpaired).  Queries ride 2 extra chunks
    (8 examples x 32 rows) into a separate small PSUM tile.
  - Host packs one dram tensor xall[128, NCH, H+2] per core (770-wide
    chunks keep every chunk 4-byte aligned for DVE 16-bit packing); col
    768 is a ones column for the softmax denominators, col 769 is pad.
  - Scores s[t] = x_t . W: fused DVE scalar_tensor_tensor per chunk
    (out = (x*1)*W, accum_out = s), with per-slot knobs to offload the
    multiply to GpSimd ("gps") or split DVE-multiply/ACT-reduce ("mix").
  - alpha = exp(s + logmask): one ACT EXP per chunk (logmask bf16,
    chunk-contiguous), bias = per-partition score column; output bf16.
  - num[j,:H] | den[j] = alpha^T @ [X | 1]: two PE matmuls (512 + 257
    cols, PSUM bank split) per chunk, bf16; slots 0,1 accumulate in PSUM
    tile A rows 0..63, slots 2,3 in tile B rows 64..127, queries in a
    [32, .] tile.  Each half is normalized (1/(den+eps)) and stored as
    soon as its slots finish, overlapping the rest of the compute.
  - b_doc / b_query shift every score in a softmax segment equally, so
    they cancel and are ignored.
"""

import numpy as np
import ml_dtypes

B, L, H = 64, 512, 768
D, S, Q = 16, 64, 32
NCORES = 8
NSLOTS = 4         # paired slots per core
MPAD = 32
NEG_BIAS = -1.0e30
DEN_EPS = 1.0e-30
BF16 = ml_dtypes.bfloat16

# score engine per slot (4 doc slots + query): "dve" | "mix" | "gps"
SCORE_ENG = ["dve", "mix", "gps", "dve", "dve"]

_compiled: dict = {}


def _slot_geometry(slot_spans):
    nts = [(sp + 127) // 128 for sp in slot_spans]
    coffs = [0]
    for nt in nts:
        coffs.append(coffs[-1] + nt)
    return nts, coffs


def _build(slot_spans):
    """Build + compile the SPMD Bass program for the given per-slot spans."""
    from contextlib import ExitStack

    import concourse.bacc as bacc
    import concourse.tile as tile
    from concourse import mybir

    f32 = mybir.dt.float32
    bf16 = mybir.dt.bfloat16
    MULT = mybir.AluOpType.mult
    ADD = mybir.AluOpType.add
    EXP = mybir.ActivationFunctionType.Exp
    COPY = mybir.ActivationFunctionType.Copy

    nts, coffs = _slot_geometry(slot_spans)
    ntsum = coffs[-1]
    NCH = ntsum + 2
    QC = [ntsum, ntsum + 1]

    nc = bacc.Bacc(
        "TRN2", target_bir_lowering=False, debug=False, num_devices=NCORES
    )
    xall = nc.dram_tensor("xall", [128, NCH, H + 2], bf16, kind="ExternalInput").ap()
    sel = nc.dram_tensor("sel", [128, NCH, MPAD], bf16, kind="ExternalInput").ap()
    wd = nc.dram_tensor("wd", [1, H], bf16, kind="ExternalInput").ap()
    wq = nc.dram_tensor("wq", [1, H], bf16, kind="ExternalInput").ap()
    out = nc.dram_tensor("out", [160, H], f32, kind="ExternalOutput").ap()

    with tile.TileContext(nc) as tc, ExitStack() as ctx:
        const = ctx.enter_context(tc.tile_pool(name="const", bufs=1))
        nump = ctx.enter_context(tc.tile_pool(name="nump", bufs=2, space="PSUM"))
        qnump = ctx.enter_context(tc.tile_pool(name="qnump", bufs=1, space="PSUM"))

        wrow_d = const.tile([1, H], bf16)
        wrow_q = const.tile([1, H], bf16)
        nc.sync.dma_start(out=wrow_d[:], in_=wd[:])
        nc.sync.dma_start(out=wrow_q[:], in_=wq[:])
        sel_t = const.tile([128, NCH, MPAD], bf16)
        nc.sync.dma_start(out=sel_t[:], in_=sel[:])
        wb_d = const.tile([128, H], bf16)
        wb_q = const.tile([128, H], bf16)
        nc.gpsimd.partition_broadcast(wb_d[:], wrow_d[:])
        nc.gpsimd.partition_broadcast(wb_q[:], wrow_q[:])

        xt = {}
        for s in range(NSLOTS):
            xt[s] = const.tile([128, nts[s], H + 2], bf16, name=f"x{s}")
        xqt = const.tile([128, 2, H + 2], bf16, name="xq")

        def load_slot(s, eng):
            eng.dma_start(
                out=xt[s][:], in_=xall[:, coffs[s] : coffs[s] + nts[s], :]
            )

        load_slot(1, nc.scalar)
        load_slot(0, nc.sync)
        load_slot(3, nc.scalar)
        load_slot(2, nc.sync)
        nc.scalar.dma_start(out=xqt[:], in_=xall[:, ntsum : ntsum + 2, :])

        scol = const.tile([128, NCH], f32)
        at = const.tile([128, NCH, MPAD], bf16)
        scratch = const.tile([128, H], bf16)
        s2 = const.tile([128, H], bf16)
        xwp = ctx.enter_context(tc.tile_pool(name="xwp", bufs=2))
        numA = nump.tile([128, 1024], f32, tag="num", name="numA")
        numB = nump.tile([128, 1024], f32, tag="num", name="numB")
        qnum = qnump.tile([32, 1024], f32, tag="qnum", name="qnum")
        doAB = const.tile([128, H], f32)
        qo = const.tile([32, H], f32)
        de = const.tile([128, 1], f32)
        rec = const.tile([128, 1], f32)
        deq = const.tile([32, 1], f32)
        recq = const.tile([32, 1], f32)

        def emit_chunk_scores(x_ap, wb, cc):
            nc.vector.scalar_tensor_tensor(
                out=scratch[:], in0=x_ap, scalar=1.0, in1=wb[:],
                op0=MULT, op1=MULT, accum_out=scol[:, cc : cc + 1],
            )

        def emit_multi_scores_eng(x_ap3, wb, cc0, nt, tt_eng):
            # multiply on tt_eng (whole slot), per-chunk accum-reduce on ACT
            xw = xwp.tile([128, nt, H], bf16, tag="xw", name=f"xw{cc0}")
            tt_eng.tensor_tensor(
                out=xw[:], in0=x_ap3,
                in1=wb[:].rearrange("p (o h) -> p o h", o=1).broadcast_to(
                    [128, nt, H]
                ),
                op=MULT,
            )
            for c in range(nt):
                nc.scalar.activation(
                    s2[:], xw[:, c, :], COPY, bias=0.0, scale=1.0,
                    accum_out=scol[:, cc0 + c : cc0 + c + 1],
                )

        def emit_chunk_alpha(cc):
            nc.scalar.activation(
                at[:, cc, :], sel_t[:, cc, :], EXP,
                bias=scol[:, cc : cc + 1], scale=1.0,
            )

        def emit_chunk_matmuls(numg, w, x_ap, cc, start, stop):
            nc.tensor.matmul(
                numg[32 * w : 32 * w + MPAD, 0:512],
                at[:, cc, :], x_ap[:, 0:512],
                start=start, stop=stop,
                tile_position=(0, 32 * w), skip_group_check=True,
            )
            nc.tensor.matmul(
                numg[32 * w : 32 * w + MPAD, 512 : H + 1],
                at[:, cc, :], x_ap[:, 512 : H + 1],
                start=start, stop=stop,
                tile_position=(0, 32 * w), skip_group_check=True,
            )

        def emit_slot(s):
            nt = nts[s]
            numg = numA if s < 2 else numB
            w = s  # slot0 rows 0..31, slot1 32..63, slot2 64..95, slot3 96..127
            eng = SCORE_ENG[s]
            if eng == "gps":
                emit_multi_scores_eng(
                    xt[s][:, :, 0:H], wb_d, coffs[s], nt, nc.gpsimd
                )
            elif eng == "mix":
                emit_multi_scores_eng(
                    xt[s][:, :, 0:H], wb_d, coffs[s], nt, nc.vector
                )
            for c in range(nt):
                cc = coffs[s] + c
                if eng == "dve":
                    emit_chunk_scores(xt[s][:, c, 0:H], wb_d, cc)
                emit_chunk_alpha(cc)
                emit_chunk_matmuls(
                    numg, w, xt[s][:, c, :], cc, c == 0, c == nt - 1
                )

        def emit_query(b):
            cc = QC[b]
            if SCORE_ENG[4] == "dve":
                emit_chunk_scores(xqt[:, b, 0:H], wb_q, cc)
            else:
                emit_multi_scores_eng(
                    xqt[:, b : b + 1, 0:H], wb_q, cc, 1, nc.vector
                )
            emit_chunk_alpha(cc)
            nc.tensor.matmul(
                qnum[0:32, 0:512], at[:, cc, :], xqt[:, b, 0:512],
                start=(b == 0), stop=(b == 1),
                tile_position=(0, 0), skip_group_check=True,
            )
            nc.tensor.matmul(
                qnum[0:32, 512 : H + 1], at[:, cc, :], xqt[:, b, 512 : H + 1],
                start=(b == 0), stop=(b == 1),
                tile_position=(0, 0), skip_group_check=True,
            )

        def finish_half(lo, hi, numg, eng):
            nc.vector.tensor_scalar(
                out=de[lo:hi], in0=numg[lo:hi, H : H + 1],
                scalar1=DEN_EPS, scalar2=None, op0=ADD,
            )
            nc.vector.reciprocal(rec[lo:hi], de[lo:hi])
            nc.scalar.activation(
                doAB[lo:hi, :], numg[lo:hi, 0:H], COPY,
                bias=0.0, scale=rec[lo:hi, 0:1],
            )
            eng.dma_start(out=out[lo:hi, :], in_=doAB[lo:hi, :])

        def finish_query(eng):
            nc.vector.tensor_scalar(
                out=deq[:], in0=qnum[:, H : H + 1],
                scalar1=DEN_EPS, scalar2=None, op0=ADD,
            )
            nc.vector.reciprocal(recq[:], deq[:])
            nc.scalar.activation(
                qo[:], qnum[:, 0:H], COPY, bias=0.0, scale=recq[:, 0:1]
            )
            eng.dma_start(out=out[128:160, :], in_=qo[:])

        emit_slot(1)
        emit_slot(0)
        finish_half(0, 64, numA, nc.sync)
        emit_slot(3)
        emit_slot(2)
        finish_half(64, 128, numB, nc.scalar)
        emit_query(0)
        emit_query(1)
        finish_query(nc.sync)

    nc.compile()
    return nc


def _prepare(query_len, seq_lens):
    """Host-side geometry: spans, pairing, slot assignment, selector masks."""
    ql = np.asarray(query_len).astype(np.int64)
    sl = np.asarray(seq_lens).astype(np.int64)
    offs = ql[:, None] + 2 + np.cumsum(sl, axis=1) - sl  # [B, D] sentence starts
    end = ql + 2 + sl.sum(axis=1)
    span = np.maximum(end, 1 + Q).astype(np.int64)
    order = np.argsort(-span, kind="stable")
    # balanced pairing: rank i with rank 63-i
    pairs = [(int(order[i]), int(order[63 - i])) for i in range(32)]
    pairspan = np.array([span[a] + span[b] for a, b in pairs])
    porder = np.argsort(-pairspan, kind="stable")
    slot_spans = tuple(int(pairspan[porder[8 * s]]) for s in range(NSLOTS))
    nts, coffs = _slot_geometry(slot_spans)
    ntsum = coffs[-1]

    # ex_map[c, s] = (eA, eB) for pair rank 8s+c
    ex_map = np.empty((NCORES, NSLOTS, 2), np.int64)
    sel_all = np.full((NCORES, 128, ntsum + 2, MPAD), NEG_BIAS, np.float32)
    for c in range(NCORES):
        for s in range(NSLOTS):
            eA, eB = pairs[int(porder[8 * s + c])]
            ex_map[c, s] = (eA, eB)
            for h, e in enumerate((eA, eB)):
                base = 0 if h == 0 else int(span[eA])
                for j in range(D):
                    ln = int(sl[e, j])
                    if ln == 0:
                        continue
                    t = base + int(offs[e, j]) + np.arange(ln)
                    sel_all[c, t % 128, coffs[s] + t // 128, 16 * h + j] = 0.0
                # query chunk: example index e2 = 2s+h -> chunk e2//4, block e2%4
                e2 = 2 * s + h
                b, k = divmod(e2, 4)
                sel_all[
                    c, 32 * k : 32 * k + int(ql[e]), ntsum + b, e2
                ] = 0.0
    return slot_spans, ex_map, sel_all, span


def kernel(hidden_states, W_doc, b_doc, W_query, b_query, query_len, seq_lens):
    hs = np.asarray(hidden_states, dtype=np.float32)
    wd = np.asarray(W_doc, np.float32).reshape(1, H).astype(BF16)
    wq = np.asarray(W_query, np.float32).reshape(1, H).astype(BF16)

    slot_spans, ex_map, sel_all, span = _prepare(query_len, seq_lens)

    nc = _compiled.get(slot_spans)
    if nc is None:
        nc = _build(slot_spans)
        _compiled[slot_spans] = nc

    nts, coffs = _slot_geometry(slot_spans)
    ntsum = coffs[-1]
    NCH = ntsum + 2

    in_maps = []
    for c in range(NCORES):
        xbuf = np.zeros((128, NCH, H + 2), np.float32)
        xbuf[:, :, H] = 1.0
        for s in range(NSLOTS):
            eA, eB = int(ex_map[c, s, 0]), int(ex_map[c, s, 1])
            spA, spB = int(span[eA]), int(span[eB])
            nt = nts[s]
            rows = np.zeros((nt * 128, H), np.float32)
            rows[:spA] = hs[eA, :spA]
            rows[spA : spA + spB] = hs[eB, :spB]
            xbuf[:, coffs[s] : coffs[s] + nt, 0:H] = (
                rows.reshape(nt, 128, H).transpose(1, 0, 2)
            )
            for h, e in enumerate((eA, eB)):
                e2 = 2 * s + h
                b, k = divmod(e2, 4)
                xbuf[32 * k : 32 * k + 32, ntsum + b, 0:H] = hs[e, 1 : 1 + Q]
        in_maps.append(
            {
                "xall": xbuf.astype(BF16),
                "sel": sel_all[c].astype(BF16),
                "wd": wd,
                "wq": wq,
            }
        )

    from concourse.bass_utils import run_bass_kernel_spmd

    res = run_bass_kernel_spmd(nc, in_maps, list(range(NCORES)))

    doc = np.empty((B, D, H), np.float32)
    qp = np.empty((B, H), np.float32)
    for c in range(NCORES):
        r = res.results[c]
        for s in range(NSLOTS):
            for h in range(2):
                e = int(ex_map[c, s, h])
                doc[e] = r["out"][32 * s + 16 * h : 32 * s + 16 * h + D, :]
                qp[e] = r["out"][128 + 2 * s + h, :]
    q_bcast = np.broadcast_to(qp[:, None, :], (B, D, H))
    return doc, q_bcast
